# revision 13
# baseline (speedup 1.0000x reference)
"""Multi-head attention (B=4, S=2048, D=1024, H=16, causal, all-valid padding)
for 8 Trainium2 NeuronCores.

Sharding: hybrid data-parallel x tensor-parallel. Core c handles batch
b = c // 2 and head-group g = c % 2 (8 heads, 512 channels each). Each core
computes its head-group's Q/K/V projections, causal attention, and the
partial output projection through its slice of Wo. The host sums the two
head-group partials per batch (the row-parallel all-reduce) and stacks
batches.

On-chip layout (per core):
  - x fed pre-transposed (D, S) so D lands on partitions for the QKV matmuls.
  - Q^T, K^T kept as [128ch, S] tiles (two 64-ch heads stacked per pair) so
    scores are computed transposed: S^T[k,q] = K_tile @ Q^T, with the two
    heads of a pair row-packed into the PE array (dk=64 each).
  - P^T = exp(S^T/8) via ACT straight out of PSUM, causal-masked by a
    precomputed bf16 mask multiply on DVE (only on diagonal tiles; fully
    masked tiles are skipped).
  - PV matmuls use a [V | ones] stationary operand (M=65): row 64 of the
    PSUM accumulator collects the softmax denominator for free while rows
    0..63 accumulate ctx^T.
  - Normalization: reciprocal_approx_fast on the denominator rows, a tiny
    K=2 selector matmul broadcasts the two heads' reciprocals across the
    128 ctx partitions, then one DVE multiply per head evicts normalized
    ctx to SBUF (bf16).
  - y = ctx_norm^T.T @ Wo^T slices; the Wo phase for q-chunk qc is emitted
    right after attention qc so it overlaps the next chunk's attention.
"""

import numpy as np
import ml_dtypes

B, S, D, H = 4, 2048, 1024, 16
DK = D // H            # 64
CH = D // 2            # 512 local channels per core (8 heads)
NPAIR = 4              # pairs of heads per core (2 heads x 64ch = 128ch tile)
SCHUNK = 512           # s-chunk (q-chunk) width
KTILE = 128            # k-tile width
NDT = D // 128         # 8 d-tiles (contraction for projections)

_BF16 = ml_dtypes.bfloat16


def _build_nc(s_len):
    import concourse.bass as bass
    import concourse.mybir as mybir
    import concourse.tile as tile
    from concourse import bacc

    f32 = mybir.dt.float32
    f32r = mybir.dt.float32r
    bf16 = mybir.dt.bfloat16
    Exp = mybir.ActivationFunctionType.Exp

    nsc = s_len // SCHUNK          # s-chunks / q-chunks
    nkt_total = s_len // KTILE     # k-tiles

    nc = bacc.Bacc("TRN2", target_bir_lowering=False, debug=False)

    xq_d = nc.dram_tensor("xqT", [D, s_len], bf16, kind="ExternalInput")
    xk_d = nc.dram_tensor("xkT", [D, s_len], bf16, kind="ExternalInput")
    xv_d = nc.dram_tensor("xvT", [D, s_len], bf16, kind="ExternalInput")
    wq_d = nc.dram_tensor("wqT", [D, CH], bf16, kind="ExternalInput")
    wk_d = nc.dram_tensor("wkT", [D, CH], bf16, kind="ExternalInput")
    wv_d = nc.dram_tensor("wvT", [D, CH], bf16, kind="ExternalInput")
    wo_d = nc.dram_tensor("woT", [CH, D], bf16, kind="ExternalInput")
    mask_d = nc.dram_tensor("masks", [4, 128, SCHUNK], bf16, kind="ExternalInput")
    y_d = nc.dram_tensor("y", [s_len, D], f32, kind="ExternalOutput")

    xq_r = xq_d[:, :].rearrange("(d p) s -> p d s", p=128)
    xk_r = xk_d[:, :].rearrange("(d p) s -> p d s", p=128)
    xv_r = xv_d[:, :].rearrange("(d p) s -> p d s", p=128)

    with tile.TileContext(nc) as tc:
        from contextlib import ExitStack

        with ExitStack() as ctx:
            const_pool = ctx.enter_context(tc.tile_pool(name="const", bufs=1))
            w_pool = ctx.enter_context(tc.tile_pool(name="weights", bufs=1))
            qt_pool = ctx.enter_context(tc.tile_pool(name="qt", bufs=NPAIR * nsc))
            kt_pool = ctx.enter_context(tc.tile_pool(name="kt", bufs=NPAIR * nsc))
            v_pool = ctx.enter_context(tc.tile_pool(name="v", bufs=nkt_total))
            ctx_pool = ctx.enter_context(tc.tile_pool(name="ctx", bufs=NPAIR * nsc))
            x_pool = ctx.enter_context(tc.tile_pool(name="x", bufs=4))
            pt_pool = ctx.enter_context(tc.tile_pool(name="pt", bufs=4))
            ev_pool = ctx.enter_context(tc.tile_pool(name="ev", bufs=4))
            y_pool = ctx.enter_context(tc.tile_pool(name="yout", bufs=3))
            qkv_ps = ctx.enter_context(
                tc.tile_pool(name="qkv_ps", bufs=2, space="PSUM"))
            st_ps = ctx.enter_context(
                tc.tile_pool(name="st_ps", bufs=2, space="PSUM"))
            ctx_ps_pool = ctx.enter_context(
                tc.tile_pool(name="ctx_ps", bufs=2, space="PSUM"))

            # critical-path first: wq + first x chunk feed the first matmuls
            wq_sb = w_pool.tile([128, NDT, CH], bf16)
            nc.sync.dma_start(
                wq_sb[:, :, :], wq_d[:, :].rearrange("(d p) c -> p d c", p=128))
            x_first = []
            for xr in (xq_r, xk_r, xv_r):
                t = x_pool.tile([128, NDT, SCHUNK], bf16, tag="x")
                nc.sync.dma_start(t[:, :, :], xr[:, :, 0:SCHUNK])
                x_first.append(t)
            wk_sb = w_pool.tile([128, NDT, CH], bf16)
            nc.sync.dma_start(
                wk_sb[:, :, :], wk_d[:, :].rearrange("(d p) c -> p d c", p=128))
            wv_sb = w_pool.tile([128, NDT, CH], bf16)
            nc.sync.dma_start(
                wv_sb[:, :, :], wv_d[:, :].rearrange("(d p) c -> p d c", p=128))
            mask_sb = const_pool.tile([128, 4, SCHUNK], bf16)
            nc.sync.dma_start(
                mask_sb[:, :, :], mask_d[:, :, :].rearrange("r p m -> p r m"))

            # stationary ones row for broadcasting a [1,512] reciprocal row
            # across a 64-partition block of a pair's ctx tile
            ones1_f = const_pool.tile([1, 64], f32)
            nc.vector.memset(ones1_f[:, :], 1.0)
            ones1_sb = const_pool.tile([1, 64], f32r)
            nc.vector.tensor_copy(ones1_sb[:, :], ones1_f[:, :])

            wo_sb = None  # loaded lazily, right after the first QKV chunk

            qt_tiles = {}
            kt_tiles = {}
            v_tiles = {}
            ctx_tiles = {}

            for sc in range(nsc):
                # ---- projections for s-chunk sc ----
                if sc == 0:
                    xq_t, xk_t, xv_t = x_first
                else:
                    xq_t = x_pool.tile([128, NDT, SCHUNK], bf16, tag="x")
                    nc.sync.dma_start(
                        xq_t[:, :, :], xq_r[:, :, sc * SCHUNK:(sc + 1) * SCHUNK])
                    xk_t = x_pool.tile([128, NDT, SCHUNK], bf16, tag="x")
                    nc.sync.dma_start(
                        xk_t[:, :, :], xk_r[:, :, sc * SCHUNK:(sc + 1) * SCHUNK])
                    xv_t = x_pool.tile([128, NDT, SCHUNK], bf16, tag="x")
                    nc.sync.dma_start(
                        xv_t[:, :, :], xv_r[:, :, sc * SCHUNK:(sc + 1) * SCHUNK])

                for m in range(NPAIR):
                    ps = qkv_ps.tile([128, SCHUNK], f32, tag="qkv")
                    for d in range(NDT):
                        nc.tensor.matmul(
                            ps[:, :],
                            lhsT=wq_sb[:, d, m * 128:(m + 1) * 128],
                            rhs=xq_t[:, d, :],
                            start=(d == 0), stop=(d == NDT - 1))
                    t = qt_pool.tile([128, SCHUNK], bf16, tag="qt",
                                     name=f"qt_{m}_{sc}")
                    nc.vector.tensor_copy(t[:, :], ps[:, :])
                    qt_tiles[(m, sc)] = t
                for m in range(NPAIR):
                    ps = qkv_ps.tile([128, SCHUNK], f32, tag="qkv")
                    for d in range(NDT):
                        nc.tensor.matmul(
                            ps[:, :],
                            lhsT=wk_sb[:, d, m * 128:(m + 1) * 128],
                            rhs=xk_t[:, d, :],
                            start=(d == 0), stop=(d == NDT - 1))
                    t = kt_pool.tile([128, SCHUNK], bf16, tag="kt",
                                     name=f"kt_{m}_{sc}")
                    nc.vector.tensor_copy(t[:, :], ps[:, :])
                    kt_tiles[(m, sc)] = t
                for ss in range(SCHUNK // 128):
                    ps = qkv_ps.tile([128, CH], f32, tag="qkv")
                    for d in range(NDT):
                        nc.tensor.matmul(
                            ps[:, :],
                            lhsT=xv_t[:, d, ss * 128:(ss + 1) * 128],
                            rhs=wv_sb[:, d, :],
                            start=(d == 0), stop=(d == NDT - 1))
                    kt_idx = sc * (SCHUNK // 128) + ss
                    # [V | ones]: col 64 of each head block feeds the softmax
                    # denominator row of the PV matmul
                    t = v_pool.tile([128, 8, 65], bf16, tag="v",
                                    name=f"v_{kt_idx}")
                    nc.vector.tensor_copy(
                        t[:, :, 0:64],
                        ps[:, :].rearrange("p (a b) -> p a b", b=64))
                    nc.vector.memset(t[:, :, 64:65], 1.0)
                    v_tiles[kt_idx] = t

                if sc == 0:
                    wo_sb = w_pool.tile([128, NPAIR, D], bf16)
                    nc.sync.dma_start(
                        wo_sb[:, :, :],
                        wo_d[:, :].rearrange("(c p) o -> p c o", p=128))

                # ---- attention for q-chunk qc = sc ----
                qc = sc
                nkt = (qc + 1) * (SCHUNK // KTILE)  # causal: k-tiles 0..nkt-1
                for pair in range(NPAIR):
                    ctx_p = [
                        ctx_ps_pool.tile([65, SCHUNK], f32, tag="ctxps",
                                         name=f"ctxp_{pair}_{qc}_{h}")
                        for h in range(2)
                    ]

                    def emit_scores(kt):
                        st = st_ps.tile([128, 2 * SCHUNK], f32, tag="st")
                        ktile = kt_tiles[(pair, kt // 4)]
                        qtile = qt_tiles[(pair, qc)]
                        for h in range(2):
                            nc.tensor.matmul(
                                st[:, h * SCHUNK:(h + 1) * SCHUNK],
                                lhsT=ktile[h * 64:(h + 1) * 64,
                                           (kt % 4) * KTILE:(kt % 4 + 1) * KTILE],
                                rhs=qtile[h * 64:(h + 1) * 64, :],
                                start=True, stop=True)
                        pt = pt_pool.tile([128, 2 * SCHUNK], bf16, tag="pt")
                        nc.scalar.activation(pt[:, :], st[:, :], Exp, scale=0.125)
                        r = kt - qc * (SCHUNK // KTILE)
                        if r >= 0:  # diagonal tile: apply causal mask
                            for h in range(2):
                                nc.vector.tensor_mul(
                                    pt[:, h * SCHUNK:(h + 1) * SCHUNK],
                                    pt[:, h * SCHUNK:(h + 1) * SCHUNK],
                                    mask_sb[:, r, :])
                        return pt

                    pt_cur = emit_scores(0)
                    for kt in range(nkt):
                        pt_next = emit_scores(kt + 1) if kt + 1 < nkt else None
                        vt = v_tiles[kt]
                        for h in range(2):
                            hl = pair * 2 + h
                            nc.tensor.matmul(
                                ctx_p[h][:, :],
                                lhsT=vt[:, hl, :],
                                rhs=pt_cur[:, h * SCHUNK:(h + 1) * SCHUNK],
                                start=(kt == 0), stop=(kt == nkt - 1))
                        pt_cur = pt_next

                    # normalization: recip of den rows -> broadcast -> scale
                    bc_sb = ev_pool.tile([128, SCHUNK], f32, tag="bcsb")
                    for h in range(2):
                        # reciprocal_approx_fast misreads PSUM at partition
                        # base 64 on HW; stage the den row through SBUF
                        den = ev_pool.tile([1, SCHUNK], f32, tag="den")
                        nc.vector.tensor_copy(den[:, :], ctx_p[h][64:65, :])
                        rec = ev_pool.tile([1, SCHUNK], f32, tag="rec")
                        nc.vector.reciprocal_approx_fast(
                            rec[:, :], den[:, :])
                        rec_r = ev_pool.tile([1, SCHUNK], f32r, tag="recr")
                        nc.vector.tensor_copy(rec_r[:, :], rec[:, :])
                        bc = qkv_ps.tile([64, SCHUNK], f32, tag="qkv")
                        nc.tensor.matmul(
                            bc[:, :],
                            lhsT=ones1_sb[:, :],
                            rhs=rec_r[:, :],
                            start=True, stop=True)
                        nc.vector.tensor_copy(
                            bc_sb[h * 64:(h + 1) * 64, :], bc[:, :])
                    t = ctx_pool.tile([128, SCHUNK], bf16, tag="ctx",
                                      name=f"ctx_{pair}_{qc}")
                    for h in range(2):
                        nc.vector.tensor_mul(
                            t[h * 64:(h + 1) * 64, :],
                            ctx_p[h][0:64, :],
                            bc_sb[h * 64:(h + 1) * 64, :])
                    ctx_tiles[(pair, qc)] = t

                # ---- output projection for this q-chunk ----
                for qt in range(qc * 4, (qc + 1) * 4):
                    for oc in range(D // 512):
                        ps = qkv_ps.tile([128, 512], f32, tag="qkv")
                        for cj in range(NPAIR):
                            nc.tensor.matmul(
                                ps[:, :],
                                lhsT=ctx_tiles[(cj, qc)][:, (qt % 4) * 128:
                                                         (qt % 4 + 1) * 128],
                                rhs=wo_sb[:, cj, oc * 512:(oc + 1) * 512],
                                start=(cj == 0), stop=(cj == NPAIR - 1))
                        yt = y_pool.tile([128, 512], f32, tag="yout")
                        nc.vector.tensor_copy(yt[:, :], ps[:, :])
                        nc.sync.dma_start(
                            y_d[qt * 128:(qt + 1) * 128,
                                oc * 512:(oc + 1) * 512],
                            yt[:, :])

    nc.finalize()
    return nc


def _make_masks():
    ki = np.arange(128)[:, None]
    qi = np.arange(SCHUNK)[None, :]
    m = np.stack([(qi >= ki + 128 * r) for r in range(4)]).astype(_BF16)
    return m


def _host_shards(x_query, x_key, x_value, Wq, Wk, Wv, Wo, s_len):
    """Per-core input dicts. Core c: batch c//2, head-group c%2."""
    masks = _make_masks()
    in_maps = []
    for c in range(8):
        b, g = c // 2, c % 2
        lo, hi = g * CH, (g + 1) * CH
        in_maps.append({
            "xqT": np.ascontiguousarray(x_query[b, :s_len].T).astype(_BF16),
            "xkT": np.ascontiguousarray(x_key[b, :s_len].T).astype(_BF16),
            "xvT": np.ascontiguousarray(x_value[b, :s_len].T).astype(_BF16),
            "wqT": np.ascontiguousarray(Wq[lo:hi, :].T).astype(_BF16),
            "wkT": np.ascontiguousarray(Wk[lo:hi, :].T).astype(_BF16),
            "wvT": np.ascontiguousarray(Wv[lo:hi, :].T).astype(_BF16),
            "woT": np.ascontiguousarray(Wo[:, lo:hi].T).astype(_BF16),
            "masks": masks,
        })
    return in_maps


_NC_CACHE = {}


def _get_nc(s_len):
    if s_len not in _NC_CACHE:
        _NC_CACHE[s_len] = _build_nc(s_len)
    return _NC_CACHE[s_len]


def kernel(x_query, x_key, x_value, attention_mask, Wq, Wk, Wv, Wo,
           _trace=False):
    from concourse.bass_utils import run_bass_kernel_spmd

    nc = _get_nc(S)
    in_maps = _host_shards(x_query, x_key, x_value, Wq, Wk, Wv, Wo, S)
    res = run_bass_kernel_spmd(nc, in_maps, core_ids=list(range(8)),
                               trace=_trace)
    y = np.empty((B, S, D), dtype=np.float32)
    for b in range(B):
        y[b] = res.results[2 * b]["y"].astype(np.float32) + \
            res.results[2 * b + 1]["y"].astype(np.float32)
    if _trace:
        return y, res
    return y


# revision 17
# speedup vs baseline: 1.2067x; 1.2067x over previous
"""Multi-head attention (B=4, S=2048, D=1024, H=16, causal, all-valid padding)
for 8 Trainium2 NeuronCores.

Sharding: hybrid data-parallel x tensor-parallel. Core c handles batch
b = c // 2 and head-group g = c % 2 (8 heads, 512 channels each). Each core
computes its head-group's Q/K/V projections, causal attention, and the
partial output projection through its slice of Wo. The host sums the two
head-group partials per batch (the row-parallel all-reduce) and stacks
batches.

On-chip layout (per core):
  - x fed pre-transposed (D, S) so D lands on partitions for the QKV matmuls.
  - Q^T, K^T kept as [128ch, S] tiles (two 64-ch heads stacked per pair) so
    scores are computed transposed: S^T[k,q] = K_tile @ Q^T, with the two
    heads of a pair row-packed into the PE array (dk=64 each).
  - P^T = exp(S^T/8) via ACT straight out of PSUM, causal-masked by a
    precomputed bf16 mask multiply on DVE (only on diagonal tiles; fully
    masked tiles are skipped).
  - PV matmuls use a [V | ones] stationary operand (M=65): row 64 of the
    PSUM accumulator collects the softmax denominator for free while rows
    0..63 accumulate ctx^T.
  - Normalization: reciprocal_approx_fast on the denominator rows, a tiny
    K=2 selector matmul broadcasts the two heads' reciprocals across the
    128 ctx partitions, then one DVE multiply per head evicts normalized
    ctx to SBUF (bf16).
  - y = ctx_norm^T.T @ Wo^T slices; the Wo phase for q-chunk qc is emitted
    right after attention qc so it overlaps the next chunk's attention.
"""

import numpy as np
import ml_dtypes

B, S, D, H = 4, 2048, 1024, 16
DK = D // H            # 64
CH = D // 2            # 512 local channels per core (8 heads)
NPAIR = 4              # pairs of heads per core (2 heads x 64ch = 128ch tile)
SCHUNK = 512           # s-chunk (q-chunk) width
KTILE = 128            # k-tile width
NDT = D // 128         # 8 d-tiles (contraction for projections)

_BF16 = ml_dtypes.bfloat16


def _build_nc(s_len):
    import concourse.bass as bass
    import concourse.mybir as mybir
    import concourse.tile as tile
    from concourse import bacc

    f32 = mybir.dt.float32
    f32r = mybir.dt.float32r
    bf16 = mybir.dt.bfloat16
    Exp = mybir.ActivationFunctionType.Exp

    nsc = s_len // SCHUNK          # s-chunks / q-chunks
    nkt_total = s_len // KTILE     # k-tiles

    nc = bacc.Bacc("TRN2", target_bir_lowering=False, debug=False)

    xq_d = nc.dram_tensor("xqT", [D, s_len], bf16, kind="ExternalInput")
    xk_d = nc.dram_tensor("xkT", [D, s_len], bf16, kind="ExternalInput")
    xv_d = nc.dram_tensor("xvT", [D, s_len], bf16, kind="ExternalInput")
    wq_d = nc.dram_tensor("wqT", [D, CH], bf16, kind="ExternalInput")
    wk_d = nc.dram_tensor("wkT", [D, CH], bf16, kind="ExternalInput")
    wv_d = nc.dram_tensor("wvT", [D, CH], bf16, kind="ExternalInput")
    wo_d = nc.dram_tensor("woT", [CH, D], bf16, kind="ExternalInput")
    mask_d = nc.dram_tensor("masks", [4, 128, SCHUNK], bf16, kind="ExternalInput")
    y_d = nc.dram_tensor("y", [s_len, D], f32, kind="ExternalOutput")

    xq_r = xq_d[:, :].rearrange("(d p) s -> p d s", p=128)
    xk_r = xk_d[:, :].rearrange("(d p) s -> p d s", p=128)
    xv_r = xv_d[:, :].rearrange("(d p) s -> p d s", p=128)

    with tile.TileContext(nc) as tc:
        from contextlib import ExitStack

        with ExitStack() as ctx:
            const_pool = ctx.enter_context(tc.tile_pool(name="const", bufs=1))
            w_pool = ctx.enter_context(tc.tile_pool(name="weights", bufs=1))
            qt_pool = ctx.enter_context(tc.tile_pool(name="qt", bufs=NPAIR * nsc))
            kt_pool = ctx.enter_context(tc.tile_pool(name="kt", bufs=NPAIR * nsc))
            v_pool = ctx.enter_context(tc.tile_pool(name="v", bufs=nkt_total))
            ctx_pool = ctx.enter_context(tc.tile_pool(name="ctx", bufs=NPAIR * nsc))
            x_pool = ctx.enter_context(tc.tile_pool(name="x", bufs=4))
            pt_pool = ctx.enter_context(tc.tile_pool(name="pt", bufs=4))
            ev_pool = ctx.enter_context(tc.tile_pool(name="ev", bufs=4))
            y_pool = ctx.enter_context(tc.tile_pool(name="yout", bufs=3))
            qkv_ps = ctx.enter_context(
                tc.tile_pool(name="qkv_ps", bufs=2, space="PSUM"))
            st_ps = ctx.enter_context(
                tc.tile_pool(name="st_ps", bufs=2, space="PSUM"))
            ctx_ps_pool = ctx.enter_context(
                tc.tile_pool(name="ctx_ps", bufs=1, space="PSUM"))
            den_ps_pool = ctx.enter_context(
                tc.tile_pool(name="den_ps", bufs=1, space="PSUM"))

            # critical-path first: wq + first x chunk feed the first matmuls
            wq_sb = w_pool.tile([128, NDT, CH], bf16)
            nc.sync.dma_start(
                wq_sb[:, :, :], wq_d[:, :].rearrange("(d p) c -> p d c", p=128))
            x_first = []
            for xr in (xq_r, xk_r, xv_r):
                t = x_pool.tile([128, NDT, SCHUNK], bf16, tag="x")
                nc.sync.dma_start(t[:, :, :], xr[:, :, 0:SCHUNK])
                x_first.append(t)
            wk_sb = w_pool.tile([128, NDT, CH], bf16)
            nc.sync.dma_start(
                wk_sb[:, :, :], wk_d[:, :].rearrange("(d p) c -> p d c", p=128))
            wv_sb = w_pool.tile([128, NDT, CH], bf16)
            nc.sync.dma_start(
                wv_sb[:, :, :], wv_d[:, :].rearrange("(d p) c -> p d c", p=128))
            mask_sb = const_pool.tile([128, 4, SCHUNK], bf16)
            nc.sync.dma_start(
                mask_sb[:, :, :], mask_d[:, :, :].rearrange("r p m -> p r m"))

            # all-ones stationary operand: the denominator matmul broadcasts
            # each head's softmax row-sums across its 64 ctx partitions
            ones_sb = const_pool.tile([128, 64], bf16)
            nc.vector.memset(ones_sb[:, :], 1.0)

            wo_sb = None  # loaded lazily, right after the first QKV chunk

            qt_tiles = {}
            kt_tiles = {}
            v_tiles = {}
            ctx_tiles = {}

            for sc in range(nsc):
                # ---- projections for s-chunk sc ----
                if sc == 0:
                    xq_t, xk_t, xv_t = x_first
                else:
                    xq_t = x_pool.tile([128, NDT, SCHUNK], bf16, tag="x")
                    nc.sync.dma_start(
                        xq_t[:, :, :], xq_r[:, :, sc * SCHUNK:(sc + 1) * SCHUNK])
                    xk_t = x_pool.tile([128, NDT, SCHUNK], bf16, tag="x")
                    nc.sync.dma_start(
                        xk_t[:, :, :], xk_r[:, :, sc * SCHUNK:(sc + 1) * SCHUNK])
                    xv_t = x_pool.tile([128, NDT, SCHUNK], bf16, tag="x")
                    nc.sync.dma_start(
                        xv_t[:, :, :], xv_r[:, :, sc * SCHUNK:(sc + 1) * SCHUNK])

                for m in range(NPAIR):
                    ps = qkv_ps.tile([128, SCHUNK], f32, tag="qkv")
                    for d in range(NDT):
                        nc.tensor.matmul(
                            ps[:, :],
                            lhsT=wq_sb[:, d, m * 128:(m + 1) * 128],
                            rhs=xq_t[:, d, :],
                            start=(d == 0), stop=(d == NDT - 1))
                    t = qt_pool.tile([128, SCHUNK], bf16, tag="qt",
                                     name=f"qt_{m}_{sc}")
                    nc.vector.tensor_copy(t[:, :], ps[:, :])
                    qt_tiles[(m, sc)] = t
                for m in range(NPAIR):
                    ps = qkv_ps.tile([128, SCHUNK], f32, tag="qkv")
                    for d in range(NDT):
                        nc.tensor.matmul(
                            ps[:, :],
                            lhsT=wk_sb[:, d, m * 128:(m + 1) * 128],
                            rhs=xk_t[:, d, :],
                            start=(d == 0), stop=(d == NDT - 1))
                    t = kt_pool.tile([128, SCHUNK], bf16, tag="kt",
                                     name=f"kt_{m}_{sc}")
                    nc.vector.tensor_copy(t[:, :], ps[:, :])
                    kt_tiles[(m, sc)] = t
                for ss in range(SCHUNK // 128):
                    ps = qkv_ps.tile([128, CH], f32, tag="qkv")
                    for d in range(NDT):
                        nc.tensor.matmul(
                            ps[:, :],
                            lhsT=xv_t[:, d, ss * 128:(ss + 1) * 128],
                            rhs=wv_sb[:, d, :],
                            start=(d == 0), stop=(d == NDT - 1))
                    kt_idx = sc * (SCHUNK // 128) + ss
                    t = v_pool.tile([128, CH], bf16, tag="v",
                                    name=f"v_{kt_idx}")
                    nc.vector.tensor_copy(t[:, :], ps[:, :])
                    v_tiles[kt_idx] = t

                if sc == 0:
                    wo_sb = w_pool.tile([128, NPAIR, D], bf16)
                    nc.sync.dma_start(
                        wo_sb[:, :, :],
                        wo_d[:, :].rearrange("(c p) o -> p c o", p=128))

                # ---- attention for q-chunk qc = sc ----
                qc = sc
                nkt = (qc + 1) * (SCHUNK // KTILE)  # causal: k-tiles 0..nkt-1
                for pair in range(NPAIR):
                    ctx_p = ctx_ps_pool.tile([128, SCHUNK], f32, tag="ctxps")
                    den_p = den_ps_pool.tile([128, SCHUNK], f32, tag="denps")

                    def tile_off(kt):
                        # diagonal tile r: columns [0, 128r) are fully masked
                        r = kt - qc * (SCHUNK // KTILE)
                        return 128 * r if r > 0 else 0

                    def emit_scores(kt):
                        off = tile_off(kt)
                        w = SCHUNK - off
                        st = st_ps.tile([128, 2 * SCHUNK], f32, tag="st")
                        ktile = kt_tiles[(pair, kt // 4)]
                        qtile = qt_tiles[(pair, qc)]
                        for h in range(2):
                            nc.tensor.matmul(
                                st[:, h * SCHUNK + off:(h + 1) * SCHUNK],
                                lhsT=ktile[h * 64:(h + 1) * 64,
                                           (kt % 4) * KTILE:(kt % 4 + 1) * KTILE],
                                rhs=qtile[h * 64:(h + 1) * 64, off:],
                                start=True, stop=True)
                        pt = pt_pool.tile([128, 2 * SCHUNK], bf16, tag="pt")
                        if off == 0:
                            nc.scalar.activation(pt[:, :], st[:, :], Exp,
                                                 scale=0.125)
                        else:
                            for h in range(2):
                                nc.scalar.activation(
                                    pt[:, h * SCHUNK + off:(h + 1) * SCHUNK],
                                    st[:, h * SCHUNK + off:(h + 1) * SCHUNK],
                                    Exp, scale=0.125)
                        r = kt - qc * (SCHUNK // KTILE)
                        if r >= 0:  # diagonal tile: apply causal mask
                            for h in range(2):
                                nc.vector.tensor_mul(
                                    pt[:, h * SCHUNK + off:(h + 1) * SCHUNK],
                                    pt[:, h * SCHUNK + off:(h + 1) * SCHUNK],
                                    mask_sb[:, r, off:])
                        return pt

                    pt_cur = emit_scores(0)
                    for kt in range(nkt):
                        pt_next = emit_scores(kt + 1) if kt + 1 < nkt else None
                        off = tile_off(kt)
                        vt = v_tiles[kt]
                        for h in range(2):
                            hl = pair * 2 + h
                            nc.tensor.matmul(
                                ctx_p[h * 64:(h + 1) * 64, off:],
                                lhsT=vt[:, hl * 64:(hl + 1) * 64],
                                rhs=pt_cur[:, h * SCHUNK + off:(h + 1) * SCHUNK],
                                start=(kt == 0), stop=(kt == nkt - 1),
                                tile_position=(0, h * 64),
                                skip_group_check=True)
                            nc.tensor.matmul(
                                den_p[h * 64:(h + 1) * 64, off:],
                                lhsT=ones_sb[:, :],
                                rhs=pt_cur[:, h * SCHUNK + off:(h + 1) * SCHUNK],
                                start=(kt == 0), stop=(kt == nkt - 1),
                                tile_position=(0, h * 64),
                                skip_group_check=True)
                        pt_cur = pt_next

                    # normalization: denominators arrive pre-broadcast across
                    # each head's 64 partitions; one recip + one multiply
                    rec = ev_pool.tile([128, SCHUNK], f32, tag="rec")
                    nc.vector.reciprocal_approx_fast(rec[:, :], den_p[:, :])
                    t = ctx_pool.tile([128, SCHUNK], bf16, tag="ctx",
                                      name=f"ctx_{pair}_{qc}")
                    nc.vector.tensor_mul(t[:, :], ctx_p[:, :], rec[:, :])
                    ctx_tiles[(pair, qc)] = t

                # ---- output projection for this q-chunk ----
                for qt in range(qc * 4, (qc + 1) * 4):
                    for oc in range(D // 512):
                        ps = qkv_ps.tile([128, 512], f32, tag="qkv")
                        for cj in range(NPAIR):
                            nc.tensor.matmul(
                                ps[:, :],
                                lhsT=ctx_tiles[(cj, qc)][:, (qt % 4) * 128:
                                                         (qt % 4 + 1) * 128],
                                rhs=wo_sb[:, cj, oc * 512:(oc + 1) * 512],
                                start=(cj == 0), stop=(cj == NPAIR - 1))
                        yt = y_pool.tile([128, 512], f32, tag="yout")
                        nc.vector.tensor_copy(yt[:, :], ps[:, :])
                        nc.sync.dma_start(
                            y_d[qt * 128:(qt + 1) * 128,
                                oc * 512:(oc + 1) * 512],
                            yt[:, :])

    nc.finalize()
    return nc


def _make_masks():
    ki = np.arange(128)[:, None]
    qi = np.arange(SCHUNK)[None, :]
    m = np.stack([(qi >= ki + 128 * r) for r in range(4)]).astype(_BF16)
    return m


def _host_shards(x_query, x_key, x_value, Wq, Wk, Wv, Wo, s_len):
    """Per-core input dicts. Core c: batch c//2, head-group c%2."""
    masks = _make_masks()
    in_maps = []
    for c in range(8):
        b, g = c // 2, c % 2
        lo, hi = g * CH, (g + 1) * CH
        in_maps.append({
            "xqT": np.ascontiguousarray(x_query[b, :s_len].T).astype(_BF16),
            "xkT": np.ascontiguousarray(x_key[b, :s_len].T).astype(_BF16),
            "xvT": np.ascontiguousarray(x_value[b, :s_len].T).astype(_BF16),
            "wqT": np.ascontiguousarray(Wq[lo:hi, :].T).astype(_BF16),
            "wkT": np.ascontiguousarray(Wk[lo:hi, :].T).astype(_BF16),
            "wvT": np.ascontiguousarray(Wv[lo:hi, :].T).astype(_BF16),
            "woT": np.ascontiguousarray(Wo[:, lo:hi].T).astype(_BF16),
            "masks": masks,
        })
    return in_maps


_NC_CACHE = {}


def _get_nc(s_len):
    if s_len not in _NC_CACHE:
        _NC_CACHE[s_len] = _build_nc(s_len)
    return _NC_CACHE[s_len]


def kernel(x_query, x_key, x_value, attention_mask, Wq, Wk, Wv, Wo,
           _trace=False):
    from concourse.bass_utils import run_bass_kernel_spmd

    nc = _get_nc(S)
    in_maps = _host_shards(x_query, x_key, x_value, Wq, Wk, Wv, Wo, S)
    res = run_bass_kernel_spmd(nc, in_maps, core_ids=list(range(8)),
                               trace=_trace)
    y = np.empty((B, S, D), dtype=np.float32)
    for b in range(B):
        y[b] = res.results[2 * b]["y"].astype(np.float32) + \
            res.results[2 * b + 1]["y"].astype(np.float32)
    if _trace:
        return y, res
    return y


# revision 18
# speedup vs baseline: 1.3572x; 1.1248x over previous
"""Multi-head attention (B=4, S=2048, D=1024, H=16, causal, all-valid padding)
for 8 Trainium2 NeuronCores.

Sharding: hybrid data-parallel x tensor-parallel. Core c handles batch
b = c // 2 and head-group g = c % 2 (8 heads, 512 channels each). Each core
computes its head-group's Q/K/V projections, causal attention, and the
partial output projection through its slice of Wo. The host sums the two
head-group partials per batch (the row-parallel all-reduce) and stacks
batches.

On-chip layout (per core):
  - x fed pre-transposed (D, S) so D lands on partitions for the QKV matmuls.
  - Q^T, K^T kept as [128ch, S] tiles (two 64-ch heads stacked per pair) so
    scores are computed transposed: S^T[k,q] = K_tile @ Q^T, with the two
    heads of a pair row-packed into the PE array (dk=64 each).
  - P^T = exp(S^T/8) via ACT straight out of PSUM, causal-masked by a
    precomputed bf16 mask multiply on DVE. Diagonal tiles only compute the
    valid column range [128r, 512); fully masked tiles are skipped.
  - ctx^T accumulates in PSUM via col-packed V-matmuls; softmax denominators
    accumulate pre-broadcast in a parallel bank via an all-ones stationary
    operand, so normalization is one reciprocal_approx_fast + one multiply.
  - y = ctx_norm^T.T @ Wo^T slices. Wo groups for chunk qc are queued and
    drip-fed into the attention of chunk qc+1 as pair-transition filler so
    the PE never idles on the softmax-normalization critical path.
"""

import numpy as np
import ml_dtypes
from collections import deque

B, S, D, H = 4, 2048, 1024, 16
DK = D // H            # 64
CH = D // 2            # 512 local channels per core (8 heads)
NPAIR = 4              # pairs of heads per core (2 heads x 64ch = 128ch tile)
SCHUNK = 512           # s-chunk (q-chunk) width
KTILE = 128            # k-tile width
NDT = D // 128         # 8 d-tiles (contraction for projections)

_BF16 = ml_dtypes.bfloat16


def _build_nc(s_len):
    import concourse.bass as bass
    import concourse.mybir as mybir
    import concourse.tile as tile
    from concourse import bacc

    f32 = mybir.dt.float32
    bf16 = mybir.dt.bfloat16
    Exp = mybir.ActivationFunctionType.Exp

    nsc = s_len // SCHUNK          # s-chunks / q-chunks
    nkt_total = s_len // KTILE     # k-tiles

    nc = bacc.Bacc("TRN2", target_bir_lowering=False, debug=False)

    xq_d = nc.dram_tensor("xqT", [D, s_len], bf16, kind="ExternalInput")
    xk_d = nc.dram_tensor("xkT", [D, s_len], bf16, kind="ExternalInput")
    xv_d = nc.dram_tensor("xvT", [D, s_len], bf16, kind="ExternalInput")
    wq_d = nc.dram_tensor("wqT", [D, CH], bf16, kind="ExternalInput")
    wk_d = nc.dram_tensor("wkT", [D, CH], bf16, kind="ExternalInput")
    wv_d = nc.dram_tensor("wvT", [D, CH], bf16, kind="ExternalInput")
    wo_d = nc.dram_tensor("woT", [CH, D], bf16, kind="ExternalInput")
    mask_d = nc.dram_tensor("masks", [4, 128, SCHUNK], bf16, kind="ExternalInput")
    y_d = nc.dram_tensor("y", [s_len, D], f32, kind="ExternalOutput")

    x_r = {
        "q": xq_d[:, :].rearrange("(d p) s -> p d s", p=128),
        "k": xk_d[:, :].rearrange("(d p) s -> p d s", p=128),
        "v": xv_d[:, :].rearrange("(d p) s -> p d s", p=128),
    }
    wq_r = wq_d[:, :].rearrange("(d p) c -> p d c", p=128)
    wk_r = wk_d[:, :].rearrange("(d p) c -> p d c", p=128)
    wv_r = wv_d[:, :].rearrange("(d p) c -> p d c", p=128)

    with tile.TileContext(nc) as tc:
        from contextlib import ExitStack

        with ExitStack() as ctx:
            const_pool = ctx.enter_context(tc.tile_pool(name="const", bufs=1))
            w_pool = ctx.enter_context(tc.tile_pool(name="weights", bufs=1))
            qt_pool = ctx.enter_context(tc.tile_pool(name="qt", bufs=NPAIR * nsc))
            kt_pool = ctx.enter_context(tc.tile_pool(name="kt", bufs=NPAIR * nsc))
            v_pool = ctx.enter_context(tc.tile_pool(name="v", bufs=nkt_total))
            ctx_pool = ctx.enter_context(tc.tile_pool(name="ctx", bufs=NPAIR * nsc))
            x_pool = ctx.enter_context(tc.tile_pool(name="x", bufs=6))
            pt_pool = ctx.enter_context(tc.tile_pool(name="pt", bufs=4))
            ev_pool = ctx.enter_context(tc.tile_pool(name="ev", bufs=4))
            y_pool = ctx.enter_context(tc.tile_pool(name="yout", bufs=3))
            qkv_ps = ctx.enter_context(
                tc.tile_pool(name="qkv_ps", bufs=2, space="PSUM"))
            st_ps = ctx.enter_context(
                tc.tile_pool(name="st_ps", bufs=2, space="PSUM"))
            ctx_ps_pool = ctx.enter_context(
                tc.tile_pool(name="ctx_ps", bufs=1, space="PSUM"))
            den_ps_pool = ctx.enter_context(
                tc.tile_pool(name="den_ps", bufs=1, space="PSUM"))

            # wq split per m-slice so the very first projection group only
            # waits on 256KB of weights + the first x chunk
            wq_sb = []
            for m in range(NPAIR):
                t = w_pool.tile([128, NDT, 128], bf16, name=f"wq_{m}")
                nc.sync.dma_start(t[:, :, :], wq_r[:, :, m * 128:(m + 1) * 128])
                wq_sb.append(t)

            x_tiles = {}

            def issue_x_dma(sc):
                for key in ("q", "k", "v"):
                    t = x_pool.tile([128, NDT, SCHUNK], bf16, tag="x",
                                    name=f"x{key}_{sc}")
                    nc.sync.dma_start(
                        t[:, :, :], x_r[key][:, :, sc * SCHUNK:(sc + 1) * SCHUNK])
                    x_tiles[(key, sc)] = t

            issue_x_dma(0)

            wk_sb = w_pool.tile([128, NDT, CH], bf16)
            nc.sync.dma_start(wk_sb[:, :, :], wk_r)
            wv_sb = w_pool.tile([128, NDT, CH], bf16)
            nc.sync.dma_start(wv_sb[:, :, :], wv_r)
            mask_sb = const_pool.tile([128, 4, SCHUNK], bf16)
            nc.sync.dma_start(
                mask_sb[:, :, :], mask_d[:, :, :].rearrange("r p m -> p r m"))
            ones_sb = const_pool.tile([128, 64], bf16)
            nc.vector.memset(ones_sb[:, :], 1.0)
            wo_sb = w_pool.tile([128, NPAIR, D], bf16)
            nc.sync.dma_start(
                wo_sb[:, :, :], wo_d[:, :].rearrange("(c p) o -> p c o", p=128))

            qt_tiles = {}
            kt_tiles = {}
            v_tiles = {}
            ctx_tiles = {}

            # deferred-work queue: (tag, closure). Attention pair starts pop
            # a couple of items as PE filler for the normalization stall.
            fill_q = deque()

            def emit_fill(n):
                for _ in range(n):
                    if not fill_q:
                        return
                    fill_q.popleft()[1]()

            def flush_tag(tag):
                while any(item[0] == tag for item in fill_q):
                    fill_q.popleft()[1]()

            def flush_all():
                while fill_q:
                    fill_q.popleft()[1]()

            def qkv_group(kind, m, sc):
                def emit():
                    if kind == "q":
                        ps = qkv_ps.tile([128, SCHUNK], f32, tag="qkv")
                        xt = x_tiles[("q", sc)]
                        for d in range(NDT):
                            nc.tensor.matmul(
                                ps[:, :], lhsT=wq_sb[m][:, d, :],
                                rhs=xt[:, d, :],
                                start=(d == 0), stop=(d == NDT - 1))
                        t = qt_pool.tile([128, SCHUNK], bf16, tag="qt",
                                         name=f"qt_{m}_{sc}")
                        nc.vector.tensor_copy(t[:, :], ps[:, :])
                        qt_tiles[(m, sc)] = t
                    elif kind == "k":
                        ps = qkv_ps.tile([128, SCHUNK], f32, tag="qkv")
                        xt = x_tiles[("k", sc)]
                        for d in range(NDT):
                            nc.tensor.matmul(
                                ps[:, :],
                                lhsT=wk_sb[:, d, m * 128:(m + 1) * 128],
                                rhs=xt[:, d, :],
                                start=(d == 0), stop=(d == NDT - 1))
                        t = kt_pool.tile([128, SCHUNK], bf16, tag="kt",
                                         name=f"kt_{m}_{sc}")
                        nc.vector.tensor_copy(t[:, :], ps[:, :])
                        kt_tiles[(m, sc)] = t
                    else:  # v
                        ps = qkv_ps.tile([128, CH], f32, tag="qkv")
                        xt = x_tiles[("v", sc)]
                        for d in range(NDT):
                            nc.tensor.matmul(
                                ps[:, :],
                                lhsT=xt[:, d, m * 128:(m + 1) * 128],
                                rhs=wv_sb[:, d, :],
                                start=(d == 0), stop=(d == NDT - 1))
                        kt_idx = sc * (SCHUNK // 128) + m
                        t = v_pool.tile([128, CH], bf16, tag="v",
                                        name=f"v_{kt_idx}")
                        nc.vector.tensor_copy(t[:, :], ps[:, :])
                        v_tiles[kt_idx] = t
                return emit

            def push_qkv(sc):
                for kind in ("q", "k", "v"):
                    for m in range(NPAIR):
                        fill_q.append((("qkv", sc), qkv_group(kind, m, sc)))

            def wo_group(qt, oc, qc):
                def emit():
                    ps = qkv_ps.tile([128, 512], f32, tag="qkv")
                    for cj in range(NPAIR):
                        nc.tensor.matmul(
                            ps[:, :],
                            lhsT=ctx_tiles[(cj, qc)][:, (qt % 4) * 128:
                                                     (qt % 4 + 1) * 128],
                            rhs=wo_sb[:, cj, oc * 512:(oc + 1) * 512],
                            start=(cj == 0), stop=(cj == NPAIR - 1))
                    yt = y_pool.tile([128, 512], f32, tag="yout")
                    nc.vector.tensor_copy(yt[:, :], ps[:, :])
                    nc.sync.dma_start(
                        y_d[qt * 128:(qt + 1) * 128, oc * 512:(oc + 1) * 512],
                        yt[:, :])
                return emit

            def push_wo(qc):
                for qt in range(qc * 4, (qc + 1) * 4):
                    for oc in range(D // 512):
                        fill_q.append((("wo", qc), wo_group(qt, oc, qc)))

            push_qkv(0)
            for sc in range(nsc):
                if sc + 1 < nsc:
                    issue_x_dma(sc + 1)
                    push_qkv(sc + 1)
                flush_tag(("qkv", sc))

                # ---- attention for q-chunk qc = sc ----
                qc = sc
                nkt = (qc + 1) * (SCHUNK // KTILE)  # causal: k-tiles 0..nkt-1
                for pair in range(NPAIR):
                    ctx_p = ctx_ps_pool.tile([128, SCHUNK], f32, tag="ctxps")
                    den_p = den_ps_pool.tile([128, SCHUNK], f32, tag="denps")

                    def tile_off(kt):
                        # diagonal tile r: columns [0, 128r) are fully masked
                        r = kt - qc * (SCHUNK // KTILE)
                        return 128 * r if r > 0 else 0

                    def emit_scores(kt):
                        off = tile_off(kt)
                        st = st_ps.tile([128, 2 * SCHUNK], f32, tag="st")
                        ktile = kt_tiles[(pair, kt // 4)]
                        qtile = qt_tiles[(pair, qc)]
                        for h in range(2):
                            nc.tensor.matmul(
                                st[:, h * SCHUNK + off:(h + 1) * SCHUNK],
                                lhsT=ktile[h * 64:(h + 1) * 64,
                                           (kt % 4) * KTILE:(kt % 4 + 1) * KTILE],
                                rhs=qtile[h * 64:(h + 1) * 64, off:],
                                start=True, stop=True)
                        pt = pt_pool.tile([128, 2 * SCHUNK], bf16, tag="pt")
                        if off == 0:
                            nc.scalar.activation(pt[:, :], st[:, :], Exp,
                                                 scale=0.125)
                        else:
                            for h in range(2):
                                nc.scalar.activation(
                                    pt[:, h * SCHUNK + off:(h + 1) * SCHUNK],
                                    st[:, h * SCHUNK + off:(h + 1) * SCHUNK],
                                    Exp, scale=0.125)
                        r = kt - qc * (SCHUNK // KTILE)
                        if r >= 0:  # diagonal tile: apply causal mask
                            for h in range(2):
                                nc.vector.tensor_mul(
                                    pt[:, h * SCHUNK + off:(h + 1) * SCHUNK],
                                    pt[:, h * SCHUNK + off:(h + 1) * SCHUNK],
                                    mask_sb[:, r, off:])
                        return pt

                    pt_cur = emit_scores(0)
                    for kt in range(nkt):
                        pt_next = emit_scores(kt + 1) if kt + 1 < nkt else None
                        if kt == 0:
                            emit_fill(2)  # PE filler over the norm stall
                        off = tile_off(kt)
                        vt = v_tiles[kt]
                        for h in range(2):
                            hl = pair * 2 + h
                            nc.tensor.matmul(
                                ctx_p[h * 64:(h + 1) * 64, off:],
                                lhsT=vt[:, hl * 64:(hl + 1) * 64],
                                rhs=pt_cur[:, h * SCHUNK + off:(h + 1) * SCHUNK],
                                start=(kt == 0), stop=(kt == nkt - 1),
                                tile_position=(0, h * 64),
                                skip_group_check=True)
                            nc.tensor.matmul(
                                den_p[h * 64:(h + 1) * 64, off:],
                                lhsT=ones_sb[:, :],
                                rhs=pt_cur[:, h * SCHUNK + off:(h + 1) * SCHUNK],
                                start=(kt == 0), stop=(kt == nkt - 1),
                                tile_position=(0, h * 64),
                                skip_group_check=True)
                        pt_cur = pt_next

                    # normalization: denominators arrive pre-broadcast across
                    # each head's 64 partitions; one recip + one multiply
                    rec = ev_pool.tile([128, SCHUNK], f32, tag="rec")
                    nc.vector.reciprocal_approx_fast(rec[:, :], den_p[:, :])
                    t = ctx_pool.tile([128, SCHUNK], bf16, tag="ctx",
                                      name=f"ctx_{pair}_{qc}")
                    nc.vector.tensor_mul(t[:, :], ctx_p[:, :], rec[:, :])
                    ctx_tiles[(pair, qc)] = t

                push_wo(qc)
            flush_all()

    nc.finalize()
    return nc


def _make_masks():
    ki = np.arange(128)[:, None]
    qi = np.arange(SCHUNK)[None, :]
    m = np.stack([(qi >= ki + 128 * r) for r in range(4)]).astype(_BF16)
    return m


def _host_shards(x_query, x_key, x_value, Wq, Wk, Wv, Wo, s_len):
    """Per-core input dicts. Core c: batch c//2, head-group c%2."""
    masks = _make_masks()
    in_maps = []
    for c in range(8):
        b, g = c // 2, c % 2
        lo, hi = g * CH, (g + 1) * CH
        in_maps.append({
            "xqT": np.ascontiguousarray(x_query[b, :s_len].T).astype(_BF16),
            "xkT": np.ascontiguousarray(x_key[b, :s_len].T).astype(_BF16),
            "xvT": np.ascontiguousarray(x_value[b, :s_len].T).astype(_BF16),
            "wqT": np.ascontiguousarray(Wq[lo:hi, :].T).astype(_BF16),
            "wkT": np.ascontiguousarray(Wk[lo:hi, :].T).astype(_BF16),
            "wvT": np.ascontiguousarray(Wv[lo:hi, :].T).astype(_BF16),
            "woT": np.ascontiguousarray(Wo[:, lo:hi].T).astype(_BF16),
            "masks": masks,
        })
    return in_maps


_NC_CACHE = {}


def _get_nc(s_len):
    if s_len not in _NC_CACHE:
        _NC_CACHE[s_len] = _build_nc(s_len)
    return _NC_CACHE[s_len]


def kernel(x_query, x_key, x_value, attention_mask, Wq, Wk, Wv, Wo,
           _trace=False):
    from concourse.bass_utils import run_bass_kernel_spmd

    nc = _get_nc(S)
    in_maps = _host_shards(x_query, x_key, x_value, Wq, Wk, Wv, Wo, S)
    res = run_bass_kernel_spmd(nc, in_maps, core_ids=list(range(8)),
                               trace=_trace)
    y = np.empty((B, S, D), dtype=np.float32)
    for b in range(B):
        y[b] = res.results[2 * b]["y"].astype(np.float32) + \
            res.results[2 * b + 1]["y"].astype(np.float32)
    if _trace:
        return y, res
    return y
